# revision 1
# baseline (speedup 1.0000x reference)
"""Batched linear-chain CRF NLL on 8 Trainium2 NeuronCores.

Strategy (data-parallel over N=64 sequences, 8 per core):
- Forward algorithm in scaled exp-space: v_t = (expT^T v_{t-1}) * exp(em_t - LAM).
  The per-step logsumexp becomes a PE matmul (block-diag expT for 2 sequences
  stacked on 128 partitions) + one DVE multiply.
- The sequential scan over L=8192 is chunk-parallelized: each sequence is cut
  into 128 chunks of 64 steps; every chunk runs as an independent chain with a
  16-step burn-in (power-iteration mixing makes the chain direction exact to
  f32 precision after ~10 steps).  log Z is reassembled from per-chain entry /
  exit column sums.
- Gold path score: per-element indirect-DMA gather for the emission term and a
  GPSIMD ap_gather from a partition-replicated flattened transition table for
  the transition term.
"""

import numpy as np
from contextlib import ExitStack

import concourse.bass as bass
import concourse.bacc as bacc
import concourse.mybir as mybir
import concourse.tile as tile
from concourse.bass import IndirectOffsetOnAxis
from concourse.bass_utils import run_bass_kernel_spmd

f32 = mybir.dt.float32
i32 = mybir.dt.int32
i16 = mybir.dt.int16
ALU = mybir.AluOpType
ACT = mybir.ActivationFunctionType
AX = mybir.AxisListType

N, C, L = 64, 64, 8192
NS = 8            # sequences per core
LAM = 4.659       # per-step log-scale shift (approx. mean log growth)
B = 16            # burn-in steps per chain
LC = 64           # chunk length (steps per chain)
NPAIR = 4         # sequence-pairs per group tile
F = NPAIR * LC    # 256 chains per 64-partition block
R = 4160          # region stride (65*64) cols per pair; data at [16, 16+4097)
NREG = NPAIR * R  # buffer cols per group


def _emit(ctx, tc, obs, tgt, trans, out, tgt_words):
    nc = tc.nc
    pool = ctx.enter_context(tc.tile_pool(name="main", bufs=1))
    vpool = [
        ctx.enter_context(tc.tile_pool(name=f"v{g}", bufs=3)) for g in range(2)
    ]
    wpool = [
        ctx.enter_context(tc.tile_pool(name=f"w{g}", bufs=2, space="PSUM"))
        for g in range(2)
    ]
    spool = ctx.enter_context(tc.tile_pool(name="stat", bufs=2, space="PSUM"))
    fpool = ctx.enter_context(tc.tile_pool(name="fin", bufs=2, space="PSUM"))

    # ---------------- constants ----------------
    # exp of transitions, block-diagonal [128, 128]
    trans2 = pool.tile([128, C], f32)
    nc.sync.dma_start(trans2[0:64, :], trans[:, :])
    nc.sync.dma_start(trans2[64:128, :], trans[:, :])
    expT2tmp = pool.tile([128, C], f32)
    nc.scalar.activation(expT2tmp[:], trans2[:], ACT.Exp)
    expT2 = pool.tile([128, 128], f32)
    nc.vector.memset(expT2[:], 0.0)
    nc.vector.tensor_copy(expT2[0:64, 0:64], expT2tmp[0:64, :])
    nc.vector.tensor_copy(expT2[64:128, 64:128], expT2tmp[64:128, :])

    # block column-sum weights [128, 2]
    ones2 = pool.tile([128, 2], f32)
    nc.vector.memset(ones2[:], 0.0)
    nc.vector.memset(ones2[0:64, 0:1], 1.0)
    nc.vector.memset(ones2[64:128, 1:2], 1.0)
    ones128 = pool.tile([128, 1], f32)
    nc.vector.memset(ones128[:], 1.0)
    ones2b = pool.tile([2, 1], f32)
    nc.vector.memset(ones2b[:], 1.0)

    # ---------------- targets ----------------
    # tgt is a flat int32 dram buffer: per seq 8192 values with stride
    # tgt_words (low word of int64 when tgt_words == 2), padded at the end.
    w = tgt_words
    seq_stride = L * w
    tgtA = pool.tile([128, 512], i32)
    tgtB = pool.tile([128, 512], i32)
    srcA = bass.AP(tgt.tensor, 0, [[16 * 512 * w, 8], [512 * w, 16], [w, 512]])
    srcB = bass.AP(tgt.tensor, w, [[16 * 512 * w, 8], [512 * w, 16], [w, 512]])
    nc.sync.dma_start(tgtA[:], srcA)
    nc.sync.dma_start(tgtB[:], srcB)

    # pair index p = a*64 + b, int16, with the 8 invalid (t=8191) pairs
    # pointed at the zero bin 4096
    ptile = pool.tile([128, 512], i16)
    nc.vector.scalar_tensor_tensor(
        ptile[:], tgtA[:], 64.0, tgtB[:], ALU.mult, ALU.add
    )
    # invalid last pair per sequence (partitions p with p%16==15) -> bin 4096
    iotaP = pool.tile([128, 1], i32)
    nc.gpsimd.iota(iotaP[:], pattern=[[0, 1]], base=0, channel_multiplier=1)
    pmod = pool.tile([128, 1], i32)
    nc.vector.tensor_scalar(pmod[:], iotaP[:], 15, None, ALU.bitwise_and)
    msk = pool.tile([128, 1], f32)
    nc.vector.tensor_scalar(msk[:], pmod[:], 15.0, None, ALU.is_equal)
    pcol = ptile[:, 511:512]
    dcol = pool.tile([128, 1], f32)
    nc.vector.tensor_scalar(dcol[:], pcol, -1.0, 4096.0, ALU.mult, ALU.add)
    nc.vector.tensor_tensor(dcol[:], dcol[:], msk[:], ALU.mult)
    nc.vector.tensor_tensor(pcol, pcol, dcol[:], ALU.add)

    # ---------------- gold_tr: replicated flat table + ap_gather ----------
    tflat = pool.tile([128, 4098], f32)
    trans_flat = bass.AP(trans.tensor, 0, [[0, 128], [1, 4096]])
    nc.gpsimd.dma_start(tflat[:, 0:4096], trans_flat)
    nc.vector.memset(tflat[:, 4096:4098], 0.0)
    gtrpool = ctx.enter_context(tc.tile_pool(name="gtrp", bufs=1))
    gtr_out = [pool.tile([128, 1], f32, name=f"gtro{r}") for r in range(2)]
    for r in range(2):
        gtr_vals = gtrpool.tile([128, 4096], f32, name="gtrv", tag="gtrv")
        nc.gpsimd.ap_gather(
            gtr_vals[:],
            tflat[:],
            ptile[:, 256 * r : 256 * (r + 1)],
            channels=128,
            num_elems=4098,
            d=1,
            num_idxs=4096,
        )
        nc.vector.reduce_sum(gtr_out[r][:], gtr_vals[:], axis=AX.X)

    # ---------------- gold_em: per-element indirect DMA gather ------------
    jio = pool.tile([128, 512], i32)
    nc.gpsimd.iota(jio[:], pattern=[[1, 512]], base=0, channel_multiplier=0)
    io512 = pool.tile([128, 1], i32)
    nc.gpsimd.iota(io512[:], pattern=[[0, 1]], base=0, channel_multiplier=512)
    io512f = pool.tile([128, 1], f32)
    nc.vector.tensor_copy(io512f[:], io512[:])
    # base_p = (p>>4)*516096 + (p&15)*512 = (p>>4)*516096 + io512 - (p>>4)*8192
    ndiv_i = pool.tile([128, 1], i32)
    nc.vector.tensor_scalar(ndiv_i[:], iotaP[:], 4, None, ALU.logical_shift_right)
    ndiv = pool.tile([128, 1], f32)
    nc.vector.tensor_copy(ndiv[:], ndiv_i[:])
    basef = pool.tile([128, 1], f32)
    nc.vector.tensor_scalar(basef[:], ndiv[:], float(C * L - 8192), None, ALU.mult)
    nc.vector.tensor_tensor(basef[:], basef[:], io512f[:], ALU.add)

    offf = pool.tile([128, 512], f32)
    nc.vector.scalar_tensor_tensor(
        offf[:], tgtA[:], float(L), jio[:], ALU.mult, ALU.add
    )
    nc.vector.tensor_scalar(offf[:], offf[:], basef[:], None, ALU.add)
    offi = pool.tile([128, 512], i32)
    nc.vector.tensor_copy(offi[:], offf[:])

    gathered = pool.tile([128, 512], f32)
    obs_flat = bass.AP(obs.tensor, 0, [[1, NS * C * L], [1, 1]])
    # HW indirect DMA consumes ONE offset per dest partition-row, so gather
    # one [128, 1] column per instruction (production tile_scatter_add pattern).
    for j in range(512):
        nc.gpsimd.indirect_dma_start(
            gathered[:, j : j + 1],
            None,
            obs_flat,
            IndirectOffsetOnAxis(ap=offi[:, j : j + 1], axis=0),
        )
    gem_p = pool.tile([128, 1], f32)
    nc.vector.reduce_sum(gem_p[:], gathered[:], axis=AX.X)

    # ---------------- emission buffers (2 time-groups) ----------------
    lamneg = pool.tile([128, 1], f32)
    nc.vector.memset(lamneg[:], -float(LAM))
    bufs = [pool.tile([128, NREG], f32, name=f"buf{g}") for g in range(2)]
    for g in range(2):
        b5 = bufs[g][:].rearrange("p (q j s) -> p q j s", q=NPAIR, j=65, s=64)
        b3 = bufs[g][:].rearrange("p (q x) -> p q x", q=NPAIR)
        t0 = g * 4096
        ncols = 4097 if g == 0 else 4096
        for blk in range(2):
            src = obs.rearrange("(q b) c t -> b c q t", b=2)[
                blk, :, :, t0 : t0 + ncols
            ]
            dst = b3[blk * 64 : (blk + 1) * 64, :, 16 : 16 + ncols]
            nc.sync.dma_start(dst, src)
        if g == 0:
            nc.vector.memset(b3[:, :, 0:16], 1.0)
        else:
            for blk in range(2):
                srcp = obs.rearrange("(q b) c t -> b c q t", b=2)[
                    blk, :, :, t0 - 15 : t0
                ]
                nc.sync.dma_start(b3[blk * 64 : (blk + 1) * 64, :, 1:16], srcp)
            nc.vector.memset(b3[:, :, 0:1], 1.0)
            nc.vector.memset(b3[:, :, 4112:4113], 1.0)
        # tail cols beyond 4113 are never read; leave them.
        # exp in place over the data region (+ pads for g=1), 4 slices
        slices = [(16, 1025), (1041, 1024), (2065, 1024), (3089, 1024)]
        if g == 1:
            slices = [(1, 15)] + [(16, 1024), (1040, 1024), (2064, 1024), (3088, 1024)]
            # note: for g=1 data region is [16, 16+4096)
        for (c0, cn) in slices:
            ap = b3[:, :, c0 : c0 + cn]
            nc.scalar.activation(ap, ap, ACT.Exp, bias=lamneg[:])

    # ---------------- forward recurrence ----------------
    stats = pool.tile([2, 5 * F], f32)  # [g1e, g1x80, g2e, g2x79, g2x80]
    STAT = {(0, 16): 0, (0, 80): 1, (1, 16): 2, (1, 79): 3, (1, 80): 4}

    v = []
    for g in range(2):
        v0 = vpool[g].tile([128, F], f32, tag="v")
        nc.vector.memset(v0[:], 1.0)
        v.append(v0)

    def colsum(g, tau, vt):
        s = STAT[(g, tau)]
        sp = spool.tile([2, F], f32, tag="s")
        nc.tensor.matmul(sp[:], ones2[:], vt[:], start=True, stop=True)
        nc.vector.tensor_copy(stats[:, s * F : (s + 1) * F], sp[:])

    for tau in range(1, 81):
        for g in range(2):
            b5 = bufs[g][:].rearrange("p (q j s) -> p q j s", q=NPAIR, j=65, s=64)
            if tau < 64:
                gsl = b5[:, :, 0:64, tau]
            else:
                gsl = b5[:, :, 1:65, tau - 64]
            wt = wpool[g].tile([128, F], f32, tag="w")
            nc.tensor.matmul(wt[:], expT2[:], v[g][:], start=True, stop=True)
            vn = vpool[g].tile([128, F], f32, tag="v")
            vn3 = vn[:].rearrange("p (q j) -> p q j", q=NPAIR)
            w3 = wt[:].rearrange("p (q j) -> p q j", q=NPAIR)
            nc.vector.tensor_tensor(vn3, w3, gsl, ALU.mult)
            v[g] = vn
            if tau == B and g == 0:
                # k=0 chains start exactly at t=0: v := exp(em_0 - LAM)
                nc.vector.tensor_copy(
                    vn3[:, :, 0:1], b5[:, :, 0:1, 16]
                )
            if (g, tau) in STAT:
                colsum(g, tau, vn)

    # ---------------- assembly ----------------
    # range-reduced ln: ACT Ln is inaccurate above ~1e3, and our column sums
    # reach 1e27.  ln(m * 2^(k-127)) = ln(m) + (k-127)*ln2 with m in [1,2).
    SF = 5 * F
    stats_i = stats[:].bitcast(i32)
    kbits = pool.tile([2, SF], i32)
    nc.vector.tensor_scalar(kbits[:], stats_i, 23, None, ALU.logical_shift_right)
    kf = pool.tile([2, SF], f32)
    nc.vector.tensor_scalar(
        kf[:], kbits[:], -127.0, 0.6931471805599453, ALU.add, ALU.mult
    )
    sbits = pool.tile([2, SF], i32)
    nc.vector.tensor_scalar(sbits[:], kbits[:], -1, 254, ALU.mult, ALU.add)
    nc.vector.tensor_scalar(sbits[:], sbits[:], 23, None, ALU.logical_shift_left)
    mant = pool.tile([2, SF], f32)
    nc.vector.tensor_tensor(mant[:], stats[:], sbits[:].bitcast(f32), ALU.mult)
    nc.scalar.activation(stats[:], mant[:], ACT.Ln)
    nc.vector.tensor_tensor(stats[:], stats[:], kf[:], ALU.add)
    ln4 = stats[:].rearrange("p (s q j) -> p s q j", s=5, q=NPAIR)

    red = [pool.tile([2, 1], f32, name=f"red{r}") for r in range(5)]
    nc.vector.reduce_sum(red[0][:], ln4[:, 1, :, :], axis=AX.XY)           # g1 exits
    nc.vector.reduce_sum(red[1][:], ln4[:, 4, :, 0:63], axis=AX.XY)       # g2 exits j<63
    nc.vector.reduce_sum(red[2][:], ln4[:, 3, :, 63:64], axis=AX.XY)      # g2 exit j=63
    nc.vector.reduce_sum(red[3][:], ln4[:, 0, :, 1:64], axis=AX.XY)       # g1 entries j>=1
    nc.vector.reduce_sum(red[4][:], ln4[:, 2, :, :], axis=AX.XY)          # g2 entries

    acc = pool.tile([2, 1], f32)
    nc.vector.tensor_tensor(acc[:], red[0][:], red[1][:], ALU.add)
    nc.vector.tensor_tensor(acc[:], acc[:], red[2][:], ALU.add)
    nc.vector.tensor_tensor(acc[:], acc[:], red[3][:], ALU.subtract)
    nc.vector.tensor_tensor(acc[:], acc[:], red[4][:], ALU.subtract)

    ps_logz = fpool.tile([1, 1], f32, tag="f")
    nc.tensor.matmul(ps_logz[:], ones2b[:], acc[:], start=True, stop=True)
    ps_gem = fpool.tile([1, 1], f32, tag="f")
    nc.tensor.matmul(ps_gem[:], ones128[:], gem_p[:], start=True, stop=True)

    gtr_p = pool.tile([128, 1], f32)
    nc.vector.tensor_tensor(gtr_p[:], gtr_out[0][:], gtr_out[1][:], ALU.add)
    ps_gtr = fpool.tile([1, 1], f32, tag="f")
    nc.tensor.matmul(ps_gtr[:], ones128[:], gtr_p[:], start=True, stop=True)
    fin = pool.tile([1, 8], f32)
    nc.vector.tensor_copy(fin[:, 0:1], ps_logz[:])
    nc.vector.tensor_copy(fin[:, 1:2], ps_gem[:])
    nc.vector.tensor_copy(fin[:, 2:3], ps_gtr[:])
    nc.vector.tensor_scalar(fin[:, 2:3], fin[:, 2:3], 1.0 / 16.0, None, ALU.mult)
    # loss_partial = logZ + NS*L*LAM - gem - gtr
    nc.vector.tensor_scalar(fin[:, 0:1], fin[:, 0:1], float(NS * L) * LAM, None, ALU.add)
    nc.vector.tensor_tensor(fin[:, 3:4], fin[:, 0:1], fin[:, 1:2], ALU.subtract)
    nc.vector.tensor_tensor(fin[:, 3:4], fin[:, 3:4], fin[:, 2:3], ALU.subtract)

    outbuf = pool.tile([1, 4], f32)
    nc.vector.tensor_copy(outbuf[:, 0:1], fin[:, 3:4])
    nc.vector.tensor_copy(outbuf[:, 1:2], fin[:, 0:1])
    nc.vector.tensor_copy(outbuf[:, 2:3], fin[:, 1:2])
    nc.vector.tensor_copy(outbuf[:, 3:4], fin[:, 2:3])
    nc.sync.dma_start(out[:, :], outbuf[:])


def build_nc(tgt_words: int):
    nc = bacc.Bacc(
        "TRN2", target_bir_lowering=False, debug=False, num_devices=8
    )
    obs = nc.dram_tensor("obs", [NS, C, L], f32, kind="ExternalInput").ap()
    tgt = nc.dram_tensor(
        "tgt", [NS * L * tgt_words + 2 * tgt_words], i32, kind="ExternalInput"
    ).ap()
    trans = nc.dram_tensor("trans", [C, C], f32, kind="ExternalInput").ap()
    out = nc.dram_tensor("out", [1, 4], f32, kind="ExternalOutput").ap()
    with tile.TileContext(nc) as tc:
        with ExitStack() as ctx:
            _emit(ctx, tc, obs, tgt, trans, out, tgt_words)
    nc.compile()
    return nc


_NC_CACHE = {}


def get_nc(tgt_words: int):
    if tgt_words not in _NC_CACHE:
        _NC_CACHE[tgt_words] = build_nc(tgt_words)
    return _NC_CACHE[tgt_words]


def make_in_maps(observes, transitions, target):
    obs = np.ascontiguousarray(np.asarray(observes), dtype=np.float32)
    trans = np.ascontiguousarray(np.asarray(transitions), dtype=np.float32)
    tgt = np.asarray(target)
    words = 2 if tgt.dtype == np.int64 else 1
    in_maps = []
    for core in range(8):
        sl = np.ascontiguousarray(tgt[core * NS : (core + 1) * NS])
        flat = sl.view(np.int32).ravel()
        flat = np.concatenate([flat, np.zeros(2 * words, np.int32)])
        in_maps.append(
            {
                "obs": np.ascontiguousarray(obs[core * NS : (core + 1) * NS]),
                "tgt": flat,
                "trans": trans,
            }
        )
    return in_maps, words


def kernel(observes, transitions, target):
    in_maps, words = make_in_maps(observes, transitions, target)
    nc = get_nc(words)
    res = run_bass_kernel_spmd(nc, in_maps, list(range(8)))
    total = sum(float(r["out"][0, 0]) for r in res.results)
    return np.float32(total / N)



# revision 33
# speedup vs baseline: 1.1897x; 1.1897x over previous
"""Batched linear-chain CRF NLL on 8 Trainium2 NeuronCores.

Strategy (data-parallel over N=64 sequences, 8 per core):
- Forward algorithm in scaled exp-space, bf16: v_t = (expT^T v_{t-1}) * exp(em_t - LAM).
  The per-step logsumexp becomes a PE matmul (block-diag expT for 2 sequences
  stacked on 128 partitions) + one DVE multiply.
- The sequential scan over L=8192 is chunk-parallelized: each sequence is cut
  into 128 chunks of 64 steps; every chunk runs as an independent chain with an
  8-step burn-in (power-iteration mixing makes the chain direction exact well
  below the error budget).  log Z is reassembled from per-chain entry / exit
  column sums with a range-reduced ln.
- Host does layout only: bf16 conversion, the chunked emission layout, and a
  blocked copy of obs for the gold-path gather (single dense DMAs on device).
- Gold path score entirely on the GPSIMD engine (no indirect DMA):
  * emissions: blocked obs copy (partition = (seq, time-block of 512),
    cols = 64c x 256 t-pairs bf16), 16 ap_gather calls (d=2 pair gathers,
    per-core wrapped index lists), strided-AP reduces on the ACT engine,
    then a diagonal-validity selection.
  * transitions: flat replicated [C*C] table + 4 ap_gather calls.
"""

import numpy as np
import ml_dtypes
from contextlib import ExitStack

import concourse.bass as bass
import concourse.bacc as bacc
import concourse.mybir as mybir
import concourse.tile as tile
from concourse.bass_utils import run_bass_kernel_spmd

f32 = mybir.dt.float32
bf16 = mybir.dt.bfloat16
i32 = mybir.dt.int32
i16 = mybir.dt.int16
ALU = mybir.AluOpType
ACT = mybir.ActivationFunctionType
AX = mybir.AxisListType

N, C, L = 64, 64, 8192
NS = 8            # sequences per core
LAM = 4.659       # per-step log-scale shift (approx. mean log growth)
B = 8             # burn-in steps per chain
LC = 64           # chunk length (steps per chain)
NPAIR = 4         # sequence-pairs per group tile
F = NPAIR * LC    # 256 chains per 64-partition block
R = 4160          # region stride (65*64) cols per pair; data at [B, B+4097)
NREG = NPAIR * R  # buffer cols per group
TPAD = 64         # int32 padding words after targets
NBF = ml_dtypes.bfloat16


def _emit(ctx, tc, emb, tbl, tgt3, tfl, trans, out):
    nc = tc.nc
    pool = ctx.enter_context(tc.tile_pool(name="main", bufs=1))
    vpool = [
        ctx.enter_context(tc.tile_pool(name=f"v{g}", bufs=3)) for g in range(2)
    ]
    wpool = [
        ctx.enter_context(tc.tile_pool(name=f"w{g}", bufs=2, space="PSUM"))
        for g in range(2)
    ]
    spool = ctx.enter_context(tc.tile_pool(name="stat", bufs=2, space="PSUM"))
    fpool = ctx.enter_context(tc.tile_pool(name="fin", bufs=2, space="PSUM"))
    gpool = ctx.enter_context(tc.tile_pool(name="gout", bufs=2))
    tpool = ctx.enter_context(tc.tile_pool(name="trout", bufs=1))

    # ---------------- iotas (Pool engine, needed early) ----------------
    iotaP = pool.tile([128, 1], i32)
    nc.gpsimd.iota(iotaP[:], pattern=[[0, 1]], base=0, channel_multiplier=1)
    # s32>>1 pattern per (tb, s32) col: 0,0,1,1,...,15,15 per 32-col block
    iota_half = pool.tile([128, 512], i32)
    nc.gpsimd.iota(iota_half[:], pattern=[[0, 16], [1, 16], [0, 2]], base=0,
                   channel_multiplier=0)
    # col index pattern for diag masks: cols (tb, b): tb = col>>1
    iota_tb2 = pool.tile([128, 32], i32)
    nc.gpsimd.iota(iota_tb2[:], pattern=[[1, 16], [0, 2]], base=0,
                   channel_multiplier=0)

    # ---------------- constants ----------------
    trans2 = pool.tile([128, C], f32)
    nc.sync.dma_start(trans2[0:64, :], trans[:, :])
    nc.sync.dma_start(trans2[64:128, :], trans[:, :])
    expT2tmp = pool.tile([128, C], bf16)
    nc.scalar.activation(expT2tmp[:], trans2[:], ACT.Exp)
    expT2 = pool.tile([128, 128], bf16)
    nc.vector.memset(expT2[:], 0.0)
    nc.vector.tensor_copy(expT2[0:64, 0:64], expT2tmp[0:64, :])
    nc.vector.tensor_copy(expT2[64:128, 64:128], expT2tmp[64:128, :])

    # block column-sum weights [128, 2] (bf16 to match moving operand)
    ones2 = pool.tile([128, 2], bf16)
    nc.vector.memset(ones2[:], 0.0)
    nc.vector.memset(ones2[0:64, 0:1], 1.0)
    nc.vector.memset(ones2[64:128, 1:2], 1.0)
    ones128 = pool.tile([128, 1], f32)
    nc.vector.memset(ones128[:], 1.0)
    ones2b = pool.tile([2, 1], f32)
    nc.vector.memset(ones2b[:], 1.0)
    lamneg = pool.tile([128, 1], f32)
    nc.vector.memset(lamneg[:], -float(LAM))

    pmod = pool.tile([128, 1], i32)  # p % 16
    nc.vector.tensor_scalar(pmod[:], iotaP[:], 15, None, ALU.bitwise_and)

    # host-laid-out targets: [ytile | ybl | ynx], one small dense DMA
    tgt3t = pool.tile([128, 1536], i32)
    nc.sync.dma_start(tgt3t[:], tgt3[:, :])
    ytile = tgt3t[:, 0:512]
    ybl = tgt3t[:, 512:1024]
    ynx = tgt3t[:, 1024:1536]

    # ---------------- emission buffer (host-laid-out, one dense DMA) ------
    bufA = pool.tile([128, 2 * NREG], bf16, name="bufA")
    nc.sync.dma_start(bufA[:], emb[:, :])
    b6 = bufA[:].rearrange("p (G q j s) -> p G q j s", G=2, q=NPAIR, j=65, s=64)
    b5s = [b6[:, g] for g in range(2)]

    # exp in place, sliced by step-within-chunk s so the recurrence can start
    # after the s<B slice instead of after the whole buffer.  Each op spans
    # BOTH groups (single bufA tile) so counter-semaphore waits stay tight.
    def _exp(ap):
        nc.scalar.activation(ap, ap, ACT.Exp, bias=lamneg[:])
    _exp(b6[:, :, :, 1:65, 0:B])          # both groups: chunks 1..64, burn-in
    _exp(b6[:, 1, :, 0:1, 1:B])           # g1: j=0 pad cols (prev-group tail)
    for s0, w in ((B, 16), (B + 16, 16), (B + 32, 16), (B + 48, 64 - B - 48)):
        _exp(b6[:, :, :, 0:64, s0 : s0 + w])  # both groups, chunks 0..63
    _exp(b6[:, 0, :, 64:65, B : B + 1])   # g0: t=4096 overlap col (exit)

    # ---------------- gold_em: blocked table + ap_gather (GPSIMD) ---------
    # table: partition (n, tb): [64c x 512t] of block tb as 16384 bf16 pairs
    table = pool.tile([128, 32768], bf16, name="gtable")
    nc.sync.dma_start(table[:], tbl[:, :])
    # idx value at (prow, (tb, s32)) = y*256 + prow*16 + (s32>>1)
    base16 = pool.tile([128, 1], f32)
    nc.vector.tensor_scalar(base16[:], pmod[:], 16.0, None, ALU.mult)
    idxtmp = pool.tile([128, 512], i32)
    nc.vector.tensor_scalar(idxtmp[:], iota_half[:], base16[:], None, ALU.add)
    idx_all = pool.tile([128, 512], i16)
    nc.vector.scalar_tensor_tensor(
        idx_all, ytile, 256.0, idxtmp[:], ALU.mult, ALU.add
    )

    # ---------------- gold_tr: replicated flat table + ap_gather ----------
    # pair index p = a*64 + b, int16; invalid last element (t=8191) -> 4096
    ptile = pool.tile([128, 512], i16)
    nc.vector.scalar_tensor_tensor(
        ptile[:], ybl, 64.0, ynx, ALU.mult, ALU.add
    )
    msk = pool.tile([128, 1], f32)
    nc.vector.tensor_scalar(msk[:], pmod[:], 15.0, None, ALU.is_equal)
    pcol = ptile[:, 511:512]
    dcol = pool.tile([128, 1], f32)
    nc.vector.tensor_scalar(dcol[:], pcol, -1.0, 4096.0, ALU.mult, ALU.add)
    nc.vector.tensor_tensor(dcol[:], dcol[:], msk[:], ALU.mult)
    nc.vector.tensor_tensor(pcol, pcol, dcol[:], ALU.add)
    tflat = pool.tile([128, 4098], f32)
    nc.sync.dma_start(tflat[:], tfl[:, :])

    # gathers on GPSIMD; per-call reduces on the ACT engine via
    # activation(Copy, accum_out=...).  Both stay off the DVE chain.
    gscr = pool.tile([128, 32], f32)  # per-call sums at cols (tb, b)
    ascr = pool.tile([128, 256], f32)  # ACT mandatory elementwise out
    gtr_scr = pool.tile([128, 4], f32)
    a3 = ascr[:, 0:256].rearrange("p (a c) -> p a c", a=16)
    for k in range(4):
        gout = gpool.tile([128, 4096], bf16, tag="gout")
        nc.gpsimd.ap_gather(
            gout[:],
            table[:],
            idx_all[:, k * 128 : (k + 1) * 128],
            channels=128,
            num_elems=16384,
            d=2,
            num_idxs=2048,
        )
        # valid element at flat col tl*1024 + 64a + 33b + 2p (a<16, b<2, p<16)
        g6 = gout[:].rearrange(
            "q (tl a c p e) -> q tl a c p e", tl=4, a=16, c=2, p=16, e=2
        )
        for tl in range(4):
            tb = k * 4 + tl
            for b in range(2):
                nc.scalar.activation(
                    a3, g6[:, tl, :, b, :, b], ACT.Copy,
                    accum_out=gscr[:, 2 * tb + b : 2 * tb + b + 1],
                )

    for r in range(4):
        trout = tpool.tile([128, 2048], f32, tag="trout")
        nc.gpsimd.ap_gather(
            trout[:],
            tflat[:],
            ptile[:, 128 * r : 128 * (r + 1)],
            channels=128,
            num_elems=4098,
            d=1,
            num_idxs=2048,
        )
        # dummy elementwise out goes into `table`, which is dead once the
        # em-gathers (earlier in the Pool queue) have run.
        nc.scalar.activation(
            table[:, 0:2048], trout[:], ACT.Copy,
            accum_out=gtr_scr[:, r : r + 1],
        )

    # ---------------- forward recurrence (bf16) ----------------
    # merged stat slices: 0 = g0 exits, 1 = g1 exits (j=63 from tau=B+63,
    # j<63 from tau=B+64), 2 = g0 entries (j=0 slot overwritten with 1.0 so
    # a full-slice reduce can subtract it), 3 = g1 entries.
    stats = pool.tile([2, 4 * F], f32)
    STAT = {(0, B): 2, (0, B + 64): 0, (1, B): 3, (1, B + 63): 1, (1, B + 64): 1}
    ln4 = stats[:].rearrange("p (s q j) -> p s q j", s=4, q=NPAIR)

    v = []
    for g in range(2):
        v0 = vpool[g].tile([128, F], bf16, tag="v")
        nc.vector.memset(v0[:], 1.0)
        v.append(v0)

    def colsum(g, tau, vt):
        s = STAT[(g, tau)]
        sp = spool.tile([2, F], f32, tag="s")
        nc.tensor.matmul(sp[:], ones2[:], vt[:], start=True, stop=True)
        sp3 = sp[:].rearrange("p (q j) -> p q j", q=NPAIR)
        if (g, tau) == (1, B + 63):
            nc.vector.tensor_copy(ln4[:, 1, :, 63:64], sp3[:, :, 63:64])
        elif (g, tau) == (1, B + 64):
            nc.vector.tensor_copy(ln4[:, 1, :, 0:63], sp3[:, :, 0:63])
        else:
            nc.vector.tensor_copy(stats[:, s * F : (s + 1) * F], sp[:])
            if (g, tau) == (0, B):
                nc.vector.memset(ln4[:, 2, :, 0:1], 1.0)

    for tau in range(1, B + 64 + 1):
        for g in range(2):
            b5 = b5s[g]
            if tau < 64:
                gsl = b5[:, :, 0:64, tau]
            else:
                gsl = b5[:, :, 1:65, tau - 64]
            wt = wpool[g].tile([128, F], f32, tag="w")
            nc.tensor.matmul(wt[:], expT2[:], v[g][:], start=True, stop=True)
            vn = vpool[g].tile([128, F], bf16, tag="v")
            vn3 = vn[:].rearrange("p (q j) -> p q j", q=NPAIR)
            w3 = wt[:].rearrange("p (q j) -> p q j", q=NPAIR)
            nc.vector.tensor_tensor(vn3, w3, gsl, ALU.mult)
            v[g] = vn
            if tau == B and g == 0:
                # k=0 chains start exactly at t=0: v := exp(em_0 - LAM)
                nc.vector.tensor_copy(vn3[:, :, 0:1], b5[:, :, 0:1, B])
            if (g, tau) in STAT:
                colsum(g, tau, vn)

    # ---------------- assembly ----------------
    # range-reduced ln: ACT Ln is inaccurate above ~1e3, and our column sums
    # reach 1e27.  ln(m * 2^(k-127)) = ln(m) + (k-127)*ln2 with m in [1,2).
    SF = 4 * F
    stats_i = stats[:].bitcast(i32)
    lnscr = pool.tile([2, 2 * SF], f32)
    kbits = lnscr[:, 0:SF].bitcast(i32)
    kf = lnscr[:, SF : 2 * SF]
    nc.vector.tensor_scalar(kbits, stats_i, 23, None, ALU.logical_shift_right)
    nc.vector.tensor_scalar(
        kf, kbits, -127.0, 0.6931471805599453, ALU.add, ALU.mult
    )
    sbits = kbits  # reuse in place
    nc.vector.tensor_scalar(sbits, kbits, -1, 254, ALU.mult, ALU.add)
    nc.vector.tensor_scalar(sbits, sbits, 23, None, ALU.logical_shift_left)
    mant = lnscr[:, 0:SF]  # overwrite sbits elementwise in place
    nc.vector.tensor_tensor(mant, stats[:], sbits.bitcast(f32), ALU.mult)
    # accum_out into ascr is a dummy: the write-write overlap forces the
    # scheduler to order this Ln AFTER the gather reduces in the ACT queue
    # (otherwise it can land before them and stall the Pool gathers until
    # the recurrence finishes).
    nc.scalar.activation(stats[:], mant, ACT.Ln, accum_out=ascr[0:2, 0:1])
    nc.vector.tensor_tensor(stats[:], stats[:], kf, ALU.add)

    acc = pool.tile([2, 2], f32)
    nc.vector.reduce_sum(acc[:, 0:1], stats[:, 0 : 2 * F], axis=AX.X)   # exits
    nc.vector.reduce_sum(acc[:, 1:2], stats[:, 2 * F : 4 * F], axis=AX.X)  # entries

    # gold_em: diagonal-validity selection of gscr then partition sum
    dmask = pool.tile([128, 32], f32)
    pmodf = pool.tile([128, 1], f32)
    nc.vector.tensor_copy(pmodf[:], pmod[:])
    nc.vector.tensor_scalar(dmask[:], iota_tb2[:], pmodf[:], None, ALU.is_equal)
    gsel = pool.tile([128, 32], f32)
    nc.vector.tensor_tensor(gsel[:], gscr[:], dmask[:], ALU.mult)

    # one matmul sums everything across partitions:
    # col0 = gold_em, col1 = gold_tr*16, col2 = logZ exits, col3 = entries
    mv = pool.tile([128, 4], f32)
    nc.vector.memset(mv[:], 0.0)
    nc.vector.reduce_sum(mv[:, 0:1], gsel[:], axis=AX.X)
    nc.vector.reduce_sum(mv[:, 1:2], gtr_scr[:], axis=AX.X)
    nc.vector.tensor_copy(mv[0:2, 2:4], acc[:])
    ps = fpool.tile([1, 4], f32, tag="f")
    nc.tensor.matmul(ps[:], ones128[:], mv[:], start=True, stop=True)

    fin = pool.tile([1, 4], f32)
    # loss = (exits - entries + NS*L*LAM) - gem - gtr/16
    nc.vector.tensor_tensor(fin[:, 1:2], ps[:, 2:3], ps[:, 3:4], ALU.subtract)
    nc.vector.tensor_scalar(fin[:, 1:2], fin[:, 1:2], float(NS * L) * LAM, None, ALU.add)
    nc.vector.scalar_tensor_tensor(
        fin[:, 2:3], ps[:, 1:2], 1.0 / 16.0, ps[:, 0:1], ALU.mult, ALU.add
    )
    nc.vector.tensor_tensor(fin[:, 0:1], fin[:, 1:2], fin[:, 2:3], ALU.subtract)
    nc.sync.dma_start(out[:, :], fin[:])


def build_nc():
    nc = bacc.Bacc(
        "TRN2", target_bir_lowering=False, debug=False, num_devices=8
    )
    emb = nc.dram_tensor("emb", [128, 2 * NREG], bf16, kind="ExternalInput").ap()
    tbl = nc.dram_tensor("tbl", [128, 32768], bf16, kind="ExternalInput").ap()
    tgt3 = nc.dram_tensor("tgt3", [128, 1536], i32, kind="ExternalInput").ap()
    tfl = nc.dram_tensor("tfl", [128, 4098], f32, kind="ExternalInput").ap()
    trans = nc.dram_tensor("trans", [C, C], f32, kind="ExternalInput").ap()
    out = nc.dram_tensor("out", [1, 4], f32, kind="ExternalOutput").ap()
    with tile.TileContext(nc) as tc:
        with ExitStack() as ctx:
            _emit(ctx, tc, emb, tbl, tgt3, tfl, trans, out)
    nc.compile()
    return nc


_NC_CACHE = {}


def get_nc(_words=None):
    if "nc" not in _NC_CACHE:
        _NC_CACHE["nc"] = build_nc()
    return _NC_CACHE["nc"]


def _host_layout(obs_b):
    """Per-core layout prep (pure data movement, no math).

    obs_b: [NS, C, L] bf16.  Returns (emb [128, 2*NREG], tbl [128, 32768])."""
    emb = np.zeros((128, 2, NPAIR, R), NBF)
    o = obs_b.reshape(NPAIR, 2, C, L)  # n = q*2 + blk
    for g in range(2):
        t0 = g * 4096
        ncols = 4097 if g == 0 else 4096
        for q in range(NPAIR):
            for blk in range(2):
                emb[blk * 64 : (blk + 1) * 64, g, q, B : B + ncols] = \
                    o[q, blk][:, t0 : t0 + ncols]
                if g == 1:
                    emb[blk * 64 : (blk + 1) * 64, 1, q, 1:B] = \
                        o[q, blk][:, t0 - B + 1 : t0]
    emb[:, 0, :, 0:B] = NBF(1.0)
    emb[:, 1, :, 0] = NBF(1.0)
    emb[:, 1, :, 64 * 64 + B] = NBF(1.0)
    tbl = np.ascontiguousarray(
        obs_b.reshape(NS, C, 16, 512).transpose(0, 2, 1, 3)
    ).reshape(128, 32768)
    return emb.reshape(128, 2 * NREG), tbl


def make_in_maps(observes, transitions, target):
    obs = np.asarray(observes).astype(NBF)
    trans = np.ascontiguousarray(np.asarray(transitions), dtype=np.float32)
    tgt = np.asarray(target).astype(np.int32)
    tfl = np.zeros((128, 4098), np.float32)
    tfl[:, 0:4096] = trans.reshape(-1)[None, :]
    in_maps = []
    for core in range(8):
        emb, tbl = _host_layout(obs[core * NS : (core + 1) * NS])
        t = np.ascontiguousarray(tgt[core * NS : (core + 1) * NS])  # [8, 8192]
        ytile = np.ascontiguousarray(
            t.reshape(NS, 16, 16, 32).transpose(0, 2, 1, 3)
        ).reshape(128, 512)
        ybl = t.reshape(128, 512)
        flat = np.concatenate([t.reshape(-1), np.zeros(TPAD, np.int32)])
        ynx = flat[1 : 1 + 65536].reshape(128, 512)
        tgt3 = np.concatenate([ytile, ybl, ynx], axis=1)
        in_maps.append(
            {"emb": emb, "tbl": tbl, "tgt3": np.ascontiguousarray(tgt3),
             "tfl": tfl, "trans": trans}
        )
    return in_maps, 1


def kernel(observes, transitions, target):
    in_maps, _ = make_in_maps(observes, transitions, target)
    nc = get_nc()
    res = run_bass_kernel_spmd(nc, in_maps, list(range(8)))
    total = sum(float(r["out"][0, 0]) for r in res.results)
    return np.float32(total / N)
